# revision 28
# baseline (speedup 1.0000x reference)
"""BottleneckLSTMCell fused kernel for 8 Trainium2 NeuronCores.

Sharding: data-parallel over batch (B=8 -> 1 image per core). Each core runs
the full cell for its image:

  phase A: xw = dw3x3(x) (+bias folded into the Wy bias); i = Wy @ [h; xw] + b
  phase B: b = dw3x3(i); four 1x1 gate matmuls; LSTM pointwise -> (ch, cc)

All matmul operands are bf16 (PSUM accumulates fp32). The tensor engine keeps
the dense 1x1 matmuls plus two depthwise chunks; the remaining five depthwise
chunks run as per-partition-scalar multiply-add chains on the vector engine
(bf16 2x mode) and gpsimd, which are otherwise idle. Phases are
software-pipelined: gates of slab n overlap dw-i of slab n+1 and phase A of
slab n+2.
"""

import sys

if '/opt/trn_rl_repo' not in sys.path:
    sys.path.insert(0, '/opt/trn_rl_repo')

import numpy as np
import ml_dtypes

import concourse.bass as bass  # noqa: F401
from concourse import bacc
import concourse.mybir as mybir
from concourse.tile import TileContext
from concourse.bass_utils import run_bass_kernel_spmd

F32 = mybir.dt.float32
BF16 = mybir.dt.bfloat16
NPBF16 = ml_dtypes.bfloat16
AF = mybir.ActivationFunctionType
ALU = mybir.AluOpType

B, CIN, CH, HW = 8, 320, 512, 64
PIX = HW * HW          # 4096
NCORES = 8
NCHUNK = 8             # spatial slabs of 8 rows (512 px)

TAPS = [(t // 3 - 1, t % 3 - 1) for t in range(9)]


def build_nc():
    nc = bacc.Bacc(None, target_bir_lowering=False, num_devices=NCORES)

    xd = nc.dram_tensor("x", (CIN, 66, 66), BF16, kind="ExternalInput")
    hd = nc.dram_tensor("h", (CH, PIX), BF16, kind="ExternalInput")
    cd = nc.dram_tensor("c", (CH, PIX), BF16, kind="ExternalInput")
    wyd = nc.dram_tensor("wy", (128, 7, 512), BF16, kind="ExternalInput")
    wybd = nc.dram_tensor("wyb", (128, 4), F32, kind="ExternalInput")
    wgd = nc.dram_tensor("wg", (128, 16, 512), BF16, kind="ExternalInput")
    # diag-packed PE dw weights: slots 0,1 = dwx chunks 0,1; slot 2 = dwx
    # chunk2 (64ch); slot 3 = dwi chunk 3
    dwxd = nc.dram_tensor("dwx", (128, 4, 1152), BF16, kind="ExternalInput")
    # per-partition tap weights: [:, 0:4] = dwi chunks, [:, 4] = dwx chunk2
    dwvd = nc.dram_tensor("dwv", (128, 5, 9), F32, kind="ExternalInput")
    # diag-packed dwi chunks 0..2 for the PE tail slab
    dwid = nc.dram_tensor("dwi", (128, 3, 1152), BF16, kind="ExternalInput")
    ccd = nc.dram_tensor("occ", (CH, PIX), BF16, kind="ExternalOutput")
    chd = nc.dram_tensor("och", (CH, PIX), BF16, kind="ExternalOutput")

    x_ap, h_ap, c_ap = xd.ap(), hd.ap(), cd.ap()
    cc_ap, ch_ap = ccd.ap(), chd.ap()

    with TileContext(nc) as tc:
        with (
            tc.tile_pool(name="persist", bufs=1) as pp,
            tc.tile_pool(name="sA", bufs=2) as sA,
            tc.tile_pool(name="sB", bufs=2) as sB,
            tc.tile_pool(name="psxw", bufs=2, space="PSUM") as psxw,
            tc.tile_pool(name="psi", bufs=3, space="PSUM") as psi,
            tc.tile_pool(name="psg", bufs=3, space="PSUM") as psg,
        ):
            dwv_t = pp.tile([128, 5, 9], F32, tag="dwv", name="dwv")

            i_pad = [pp.tile([128, 66, 66], BF16, tag=f"ipad{m}",
                             name=f"ipad{m}") for m in range(4)]
            for m in range(4):
                nc.vector.memset(i_pad[m][:, 0, :], 0.0)
                nc.vector.memset(i_pad[m][:, 65, :], 0.0)
                nc.vector.memset(i_pad[m][:, :, 0], 0.0)
                nc.vector.memset(i_pad[m][:, :, 65], 0.0)

            # PE diag weights -- dwx slots first (startup critical); the
            # dwi3 slot is not needed until the first emit_dwi
            dwx_t = pp.tile([128, 4, 1152], BF16, tag="dwx", name="dwx")
            with tc.high_priority():
                for _s in range(3):
                    nc.sync.dma_start(out=dwx_t[:, _s, :],
                                      in_=dwxd.ap()[:, _s, :])
            nc.scalar.dma_start(out=dwx_t[:, 3, :], in_=dwxd.ap()[:, 3, :])

            def emit_slab_inputs(n, q=None):
                q = q or nc.sync
                r0 = 8 * n
                xps = []
                for ci in range(3):
                    pc = 128 if ci < 2 else 64
                    xp = sA.tile([128, 10, 66], BF16, tag=f"xpad{ci}",
                                 name=f"xpad{ci}")
                    q.dma_start(
                        out=xp[:pc, :, :],
                        in_=x_ap[128 * ci:128 * ci + pc, r0:r0 + 10, :])
                    xps.append(xp)
                ht = sA.tile([128, 4, 512], BF16, tag="h", name="h")
                q.dma_start(
                    out=ht[:],
                    in_=h_ap[:, 512 * n:512 * (n + 1)].rearrange(
                        "(k p) x -> p k x", p=128))
                return ht, xps

            with tc.high_priority():
                ins = {0: emit_slab_inputs(0, q=nc.gpsimd)}
            # dwv is not needed until the first DVE dwi chain (step 1)
            nc.gpsimd.dma_start(out=dwv_t[:], in_=dwvd.ap())

            # bulk weights off the critical queue (scalar engine is idle now)
            wy_t = pp.tile([128, 7, 512], BF16, tag="wy", name="wy")
            nc.scalar.dma_start(out=wy_t[:], in_=wyd.ap())
            wyb_t = pp.tile([128, 4], F32, tag="wyb", name="wyb")
            nc.scalar.dma_start(out=wyb_t[:], in_=wybd.ap())
            wg_t = pp.tile([128, 16, 512], BF16, tag="wg", name="wg")
            nc.scalar.dma_start(out=wg_t[:], in_=wgd.ap())
            dwi_t = pp.tile([128, 3, 1152], BF16, tag="dwi", name="dwi")
            nc.scalar.dma_start(out=dwi_t[:], in_=dwid.ap())

            ins[1] = emit_slab_inputs(1)

            def dw_chain(eng, out_t, scr_t, win_fn, w_col, np_=128):
                """9-tap depthwise on an elementwise engine.
                out_t/scr_t: [np_,8,64] bf16 ping-pong; win_fn(dy,dx)->AP."""
                for t, (dy, dx) in enumerate(TAPS):
                    win = win_fn(dy, dx)
                    if t == 0:
                        eng.tensor_scalar(
                            out=out_t[:np_, :, :], in0=win,
                            scalar1=w_col(t), scalar2=None, op0=ALU.mult)
                    else:
                        dst, src = (out_t, scr_t) if t % 2 == 0 else \
                                   (scr_t, out_t)
                        eng.scalar_tensor_tensor(
                            out=dst[:np_, :, :], in0=win, scalar=w_col(t),
                            in1=src[:np_, :, :], op0=ALU.mult, op1=ALU.add)

            def pe_dw_chunk(wslot, img, out_tag, pc=128, wt=None):
                wt = wt if wt is not None else dwx_t
                ps = psxw.tile([128, 8, 64], F32, tag="psxw", name="psxw")
                for t, (dy, dx) in enumerate(TAPS):
                    nc.tensor.matmul(
                        ps[:pc, :, :],
                        wt[:pc, wslot, 128 * t:128 * t + pc],
                        img[:pc, 1 + dy:9 + dy, 1 + dx:65 + dx],
                        start=(t == 0), stop=(t == 8))
                out = sA.tile([128, 8, 64], BF16, tag=out_tag, name=out_tag)
                nc.scalar.copy(out[:pc, :, :], ps[:pc, :, :])
                return out

            def emit_A(n, ht, xps):
                r0 = 8 * n
                early = {}
                if n == 0:
                    # startup: h partials need only h0+wy, which arrive well
                    # before the x slabs and dw weights
                    for m in range(2):
                        ps = psi.tile([128, 512], F32, tag="psi", name="psi")
                        for k in range(4):
                            nc.tensor.matmul(
                                ps[:, :], wy_t[:, k, 128 * m:128 * (m + 1)],
                                ht[:, k, :], start=(k == 0), stop=False)
                        early[m] = ps
                xw01 = [pe_dw_chunk(ci, xps[ci], f"xw{ci}") for ci in range(2)]
                xw2 = pe_dw_chunk(2, xps[2], "xw2", pc=64)

                for m in range(4):
                    if m in early:
                        ps = early[m]
                    else:
                        ps = psi.tile([128, 512], F32, tag="psi", name="psi")
                        for k in range(4):  # h chunks first (ready earlier)
                            nc.tensor.matmul(
                                ps[:, :], wy_t[:, k, 128 * m:128 * (m + 1)],
                                ht[:, k, :], start=(k == 0), stop=False)
                    nc.tensor.matmul(
                        ps[:, :], wy_t[:, 4, 128 * m:128 * (m + 1)],
                        xw01[0][:, :, :], start=False, stop=False)
                    nc.tensor.matmul(
                        ps[:, :], wy_t[:, 5, 128 * m:128 * (m + 1)],
                        xw01[1][:, :, :], start=False, stop=False)
                    nc.tensor.matmul(
                        ps[:, :], wy_t[:64, 6, 128 * m:128 * (m + 1)],
                        xw2[0:64, :, :], start=False, stop=True)
                    nc.scalar.activation(
                        i_pad[m][:, 1 + r0:9 + r0, 1:65], ps[:, :],
                        AF.Identity, bias=wyb_t[:, m:m + 1], scale=1.0)

            def emit_dwi(n):
                r0 = 8 * n
                b_sb = []
                for ci in range(3):
                    if n == NCHUNK - 1:  # tail slab: PE is idle, DVE is not
                        b_sb.append(pe_dw_chunk(
                            ci, i_pad[ci][:, r0:r0 + 10, :], f"b{ci}",
                            wt=dwi_t))
                        continue
                    bt = sB.tile([128, 8, 64], BF16, tag=f"b{ci}",
                                 name=f"b{ci}")
                    scr = sB.tile([128, 8, 64], BF16, tag=f"bs{ci}",
                                  name=f"bs{ci}")
                    dw_chain(
                        nc.vector, bt, scr,
                        lambda dy, dx, _c=ci: i_pad[_c][
                            :, 1 + r0 + dy:9 + r0 + dy, 1 + dx:65 + dx],
                        lambda t, _c=ci: dwv_t[:, _c, t:t + 1])
                    b_sb.append(bt)
                # chunk 3 on PE (diag matmul, weight slot 3)
                b_sb.append(pe_dw_chunk(3, i_pad[3][:, r0:r0 + 10, :], "b3"))
                return b_sb

            def emit_gates(n, b_sb):
                c_t = sB.tile([128, 4, 512], BF16, tag="c", name="c")
                nc.sync.dma_start(
                    out=c_t[:],
                    in_=c_ap[:, 512 * n:512 * (n + 1)].rearrange(
                        "(k p) x -> p k x", p=128))
                occ_t = sB.tile([128, 4, 512], BF16, tag="occ", name="occ")
                och_t = sB.tile([128, 4, 512], BF16, tag="och", name="och")
                for m in range(4):
                    sig = []
                    for g in range(4):  # 0=i 1=f 2=c 3=o
                        ps = psg.tile([128, 512], F32, tag="psg", name="psg")
                        for k in range(4):
                            nc.tensor.matmul(
                                ps[:, :],
                                wg_t[:, 4 * g + k, 128 * m:128 * (m + 1)],
                                b_sb[k][:, :, :],
                                start=(k == 0), stop=(k == 3))
                        st = sB.tile([128, 512], BF16, tag=f"sg{g}",
                                     name=f"sg{g}")
                        nc.scalar.activation(
                            st[:, :], ps[:, :],
                            AF.Relu if g == 2 else AF.Sigmoid)
                        sig.append(st)

                    u1 = sB.tile([128, 512], BF16, tag="u1", name="u1")
                    nc.vector.tensor_mul(u1[:, :], sig[1][:, :], c_t[:, m, :])
                    u2 = sB.tile([128, 512], BF16, tag="u2", name="u2")
                    nc.vector.scalar_tensor_tensor(
                        out=u2[:, :], in0=sig[2][:, :], scalar=6.0,
                        in1=sig[0][:, :], op0=ALU.min, op1=ALU.mult)
                    nc.vector.tensor_add(occ_t[:, m, :], u1[:, :], u2[:, :])
                    rcc = sB.tile([128, 512], BF16, tag="rcc", name="rcc")
                    nc.vector.tensor_scalar(
                        out=rcc[:, :], in0=occ_t[:, m, :],
                        scalar1=0.0, scalar2=6.0, op0=ALU.max, op1=ALU.min)
                    nc.vector.tensor_mul(och_t[:, m, :], rcc[:, :],
                                         sig[3][:, :])
                    if n == NCHUNK - 1:
                        # tail slab: drain each m as soon as it is ready
                        nc.sync.dma_start(
                            out=cc_ap[128 * m:128 * (m + 1),
                                      512 * n:512 * (n + 1)],
                            in_=occ_t[:, m, :])
                        nc.sync.dma_start(
                            out=ch_ap[128 * m:128 * (m + 1),
                                      512 * n:512 * (n + 1)],
                            in_=och_t[:, m, :])
                if n < NCHUNK - 1:
                    nc.sync.dma_start(
                        out=cc_ap[:, 512 * n:512 * (n + 1)].rearrange(
                            "(k p) x -> p k x", p=128),
                        in_=occ_t[:])
                    nc.sync.dma_start(
                        out=ch_ap[:, 512 * n:512 * (n + 1)].rearrange(
                            "(k p) x -> p k x", p=128),
                        in_=och_t[:])

            # ---- software pipeline ----
            bq = {}
            for n in range(NCHUNK + 2):
                if n < NCHUNK:
                    if n not in ins:
                        ins[n] = emit_slab_inputs(n)
                    emit_A(n, *ins.pop(n))
                    if n + 1 < NCHUNK and n + 1 not in ins:
                        ins[n + 1] = emit_slab_inputs(n + 1)
                if 1 <= n <= NCHUNK:
                    bq[n - 1] = emit_dwi(n - 1)
                if n >= 2:
                    emit_gates(n - 2, bq.pop(n - 2))

    nc.compile()
    return nc


def pack_weights(W_dw, W_dwb, Wy, Wy_b, Wi, Wbi, Wbf, Wbc, Wbo):
    WyT = Wy[:, :, 0, 0].T.astype(np.float32)  # (832, 512) lhsT
    wy = np.zeros((128, 7, 512), np.float32)
    for k in range(4):  # h chunks first
        wy[:, k, :] = WyT[320 + 128 * k:320 + 128 * (k + 1), :]
    for k in range(2):
        wy[:, 4 + k, :] = WyT[128 * k:128 * (k + 1), :]
    wy[:64, 6, :] = WyT[256:320, :]

    wyb = (Wy_b + Wy[:, :320, 0, 0] @ W_dwb).astype(np.float32)
    wyb = wyb.reshape(4, 128).T.copy()

    wdx = W_dw[:, 0].reshape(CIN, 9)
    wdi = Wi[:, 0].reshape(CH, 9)
    dwx = np.zeros((128, 4, 1152), np.float32)
    for slot, w in enumerate([wdx[0:128], wdx[128:256], wdx[256:320],
                              wdi[384:512]]):
        pc = w.shape[0]
        for t in range(9):
            idx = np.arange(pc)
            dwx[idx, slot, 128 * t + idx] = w[idx, t]

    dwv = np.zeros((128, 5, 9), np.float32)
    for ci in range(4):
        dwv[:, ci, :] = wdi[128 * ci:128 * (ci + 1), :]
    dwi = np.zeros((128, 3, 1152), np.float32)
    for ci in range(3):
        for t in range(9):
            idx = np.arange(128)
            dwi[idx, ci, 128 * t + idx] = wdi[128 * ci + idx, t]

    wg = np.zeros((128, 16, 512), np.float32)
    for g, W in enumerate([Wbi, Wbf, Wbc, Wbo]):
        lhsT = W[:, :, 0, 0].T.astype(np.float32)  # (512 in, 512 out)
        for k in range(4):
            wg[:, 4 * g + k, :] = lhsT[128 * k:128 * (k + 1), :]

    bf = lambda a: np.ascontiguousarray(a).astype(NPBF16)
    return {
        "wy": bf(wy), "wyb": np.ascontiguousarray(wyb), "wg": bf(wg),
        "dwx": bf(dwx), "dwi": bf(dwi),
        "dwv": np.ascontiguousarray(dwv, dtype=np.float32),
    }


_CACHE = {}


def _get_nc():
    if "nc" not in _CACHE:
        _CACHE["nc"] = build_nc()
    return _CACHE["nc"]


def run(inputs, trace=False, tmpdir=None):
    """inputs: dict as from setup_inputs(). Returns ((ch, cc), results_obj)."""
    inp = {k: np.asarray(v, np.float32) for k, v in inputs.items()}
    packed = pack_weights(
        inp["W_dw"], inp["W_dwb"], inp["Wy"], inp["Wy_b"], inp["Wi"],
        inp["Wbi"], inp["Wbf"], inp["Wbc"], inp["Wbo"],
    )
    xpad_host = np.zeros((B, CIN, 66, 66), NPBF16)
    xpad_host[:, :, 1:65, 1:65] = inp["x"].astype(NPBF16)
    h_host = inp["h"].reshape(B, CH, PIX).astype(NPBF16)
    c_host = inp["c"].reshape(B, CH, PIX).astype(NPBF16)
    in_maps = []
    for b in range(B):
        in_maps.append({
            "x": xpad_host[b],
            "h": np.ascontiguousarray(h_host[b]),
            "c": np.ascontiguousarray(c_host[b]),
            **packed,
        })
    nc = _get_nc()
    kwargs = {}
    if trace:
        _enable_trace_hooks()
        kwargs = dict(trace=True, trace_cores=[0])
        if tmpdir:
            kwargs["tmpdir"] = tmpdir
    res = run_bass_kernel_spmd(nc, in_maps, core_ids=list(range(NCORES)), **kwargs)
    ch = np.stack([res.results[b]["och"].astype(np.float32).reshape(CH, HW, HW)
                   for b in range(B)])
    cc = np.stack([res.results[b]["occ"].astype(np.float32).reshape(CH, HW, HW)
                   for b in range(B)])
    return (ch, cc), res


def kernel(**inputs):
    (ch, cc), _ = run(inputs, trace=False)
    return ch, cc


# ---------- optional NTFF tracing support (test harness only) ----------

def _enable_trace_hooks():
    import types, ctypes, contextlib
    if "antenv.axon_hooks" in sys.modules:
        return
    import concourse.bass_utils as bass_utils

    def _ntff_profile_via_ctypes(so_path):
        lib = ctypes.CDLL(so_path)
        if not hasattr(lib, "axon_start_nrt_profile"):
            return None
        lib.axon_start_nrt_profile.argtypes = [
            ctypes.POINTER(ctypes.c_int64), ctypes.c_size_t]
        lib.axon_start_nrt_profile.restype = ctypes.c_int64
        lib.axon_stop_nrt_profile.argtypes = [ctypes.c_char_p]
        lib.axon_stop_nrt_profile.restype = ctypes.c_int64

        @contextlib.contextmanager
        def _hook(output_dir, device_ids):
            import jax
            jax.devices()
            if device_ids:
                ids = (ctypes.c_int64 * len(device_ids))(*device_ids)
                rc = lib.axon_start_nrt_profile(ids, len(device_ids))
            else:
                rc = lib.axon_start_nrt_profile(None, 0)
            if rc != 0:
                raise RuntimeError(f"axon_start_nrt_profile rc={rc}")
            try:
                yield
            finally:
                lib.axon_stop_nrt_profile(str(output_dir).encode())
        return _hook

    hook = _ntff_profile_via_ctypes("/opt/axon/libaxon_pjrt.so")
    mod = types.ModuleType("antenv.axon_hooks")
    mod.get_axon_ntff_profile_hook = lambda: hook
    mod.set_axon_ntff_profile_hook = lambda h: None
    sys.modules["antenv.axon_hooks"] = mod
    bass_utils.upload_artifacts = lambda tmpdir: "local://" + str(tmpdir)


# revision 29
# speedup vs baseline: 1.2111x; 1.2111x over previous
"""BottleneckLSTMCell fused kernel for 8 Trainium2 NeuronCores.

Sharding: data-parallel over batch (B=8 -> 1 image per core). Each core runs
the full cell for its image:

  phase A: xw = dw3x3(x) (+bias folded into the Wy bias); i = Wy @ [h; xw] + b
  phase B: b = dw3x3(i); four 1x1 gate matmuls; LSTM pointwise -> (ch, cc)

All matmul operands are bf16 (PSUM accumulates fp32). The tensor engine keeps
the dense 1x1 matmuls plus two depthwise chunks; the remaining five depthwise
chunks run as per-partition-scalar multiply-add chains on the vector engine
(bf16 2x mode) and gpsimd, which are otherwise idle. Phases are
software-pipelined: gates of slab n overlap dw-i of slab n+1 and phase A of
slab n+2.
"""

import sys

if '/opt/trn_rl_repo' not in sys.path:
    sys.path.insert(0, '/opt/trn_rl_repo')

import numpy as np
import ml_dtypes

import concourse.bass as bass  # noqa: F401
from concourse import bacc
import concourse.mybir as mybir
from concourse.tile import TileContext
from concourse.bass_utils import run_bass_kernel_spmd

F32 = mybir.dt.float32
BF16 = mybir.dt.bfloat16
NPBF16 = ml_dtypes.bfloat16
AF = mybir.ActivationFunctionType
ALU = mybir.AluOpType

B, CIN, CH, HW = 8, 320, 512, 64
PIX = HW * HW          # 4096
NCORES = 8
NCHUNK = 8             # spatial slabs of 8 rows (512 px)

TAPS = [(t // 3 - 1, t % 3 - 1) for t in range(9)]


def build_nc():
    nc = bacc.Bacc(None, target_bir_lowering=False, num_devices=NCORES)

    xd = nc.dram_tensor("x", (CIN, 66, 66), BF16, kind="ExternalInput")
    hd = nc.dram_tensor("h", (CH, PIX), BF16, kind="ExternalInput")
    cd = nc.dram_tensor("c", (CH, PIX), BF16, kind="ExternalInput")
    wyd = nc.dram_tensor("wy", (128, 7, 512), BF16, kind="ExternalInput")
    wybd = nc.dram_tensor("wyb", (128, 4), F32, kind="ExternalInput")
    wgd = nc.dram_tensor("wg", (128, 16, 512), BF16, kind="ExternalInput")
    # diag-packed PE dw weights: slots 0,1 = dwx chunks 0,1; slot 2 = dwx
    # chunk2 (64ch); slot 3 = dwi chunk 3
    dwxd = nc.dram_tensor("dwx", (128, 4, 1152), BF16, kind="ExternalInput")
    # per-partition tap weights: [:, 0:4] = dwi chunks, [:, 4] = dwx chunk2
    dwvd = nc.dram_tensor("dwv", (128, 5, 9), F32, kind="ExternalInput")
    # diag-packed dwi chunks 0..2 for the PE tail slab
    dwid = nc.dram_tensor("dwi", (128, 3, 1152), BF16, kind="ExternalInput")
    ccd = nc.dram_tensor("occ", (CH, PIX), BF16, kind="ExternalOutput")
    chd = nc.dram_tensor("och", (CH, PIX), BF16, kind="ExternalOutput")

    x_ap, h_ap, c_ap = xd.ap(), hd.ap(), cd.ap()
    cc_ap, ch_ap = ccd.ap(), chd.ap()

    with TileContext(nc) as tc:
        with (
            tc.tile_pool(name="persist", bufs=1) as pp,
            tc.tile_pool(name="sA", bufs=2) as sA,
            tc.tile_pool(name="sB", bufs=2) as sB,
            tc.tile_pool(name="psxw", bufs=2, space="PSUM") as psxw,
            tc.tile_pool(name="psi", bufs=3, space="PSUM") as psi,
            tc.tile_pool(name="psg", bufs=3, space="PSUM") as psg,
        ):
            # per-partition dw tap weights -- first on the vector queue so
            # the slab-0 DVE chains are unblocked immediately
            dwv_t = pp.tile([128, 5, 9], F32, tag="dwv", name="dwv")
            nc.gpsimd.dma_start(out=dwv_t[:], in_=dwvd.ap())

            i_pad = [pp.tile([128, 66, 66], BF16, tag=f"ipad{m}",
                             name=f"ipad{m}") for m in range(4)]
            for m in range(4):
                nc.vector.memset(i_pad[m][:, 0, :], 0.0)
                nc.vector.memset(i_pad[m][:, 65, :], 0.0)
                nc.vector.memset(i_pad[m][:, :, 0], 0.0)
                nc.vector.memset(i_pad[m][:, :, 65], 0.0)

            # PE diag weights -- dwx slots first (startup critical); the
            # dwi3 slot is not needed until the first emit_dwi
            dwx_t = pp.tile([128, 4, 1152], BF16, tag="dwx", name="dwx")
            with tc.high_priority():
                for _s in range(3):
                    nc.sync.dma_start(out=dwx_t[:, _s, :],
                                      in_=dwxd.ap()[:, _s, :])
            nc.scalar.dma_start(out=dwx_t[:, 3, :], in_=dwxd.ap()[:, 3, :])

            def emit_slab_inputs(n, q=None):
                q = q or nc.sync
                r0 = 8 * n
                xps = []
                for ci in range(3):
                    pc = 128 if ci < 2 else 64
                    xp = sA.tile([128, 10, 66], BF16, tag=f"xpad{ci}",
                                 name=f"xpad{ci}")
                    q.dma_start(
                        out=xp[:pc, :, :],
                        in_=x_ap[128 * ci:128 * ci + pc, r0:r0 + 10, :])
                    xps.append(xp)
                ht = sA.tile([128, 4, 512], BF16, tag="h", name="h")
                q.dma_start(
                    out=ht[:],
                    in_=h_ap[:, 512 * n:512 * (n + 1)].rearrange(
                        "(k p) x -> p k x", p=128))
                return ht, xps

            with tc.high_priority():
                ins = {0: emit_slab_inputs(0, q=nc.gpsimd)}

            # bulk weights off the critical queue (scalar engine is idle now)
            wy_t = pp.tile([128, 7, 512], BF16, tag="wy", name="wy")
            nc.scalar.dma_start(out=wy_t[:], in_=wyd.ap())
            wyb_t = pp.tile([128, 4], F32, tag="wyb", name="wyb")
            nc.scalar.dma_start(out=wyb_t[:], in_=wybd.ap())
            wg_t = pp.tile([128, 16, 512], BF16, tag="wg", name="wg")
            nc.scalar.dma_start(out=wg_t[:], in_=wgd.ap())
            dwi_t = pp.tile([128, 3, 1152], BF16, tag="dwi", name="dwi")
            nc.scalar.dma_start(out=dwi_t[:], in_=dwid.ap())

            ins[1] = emit_slab_inputs(1)

            def dw_chain(eng, out_t, scr_t, win_fn, w_col, np_=128):
                """9-tap depthwise on an elementwise engine.
                out_t/scr_t: [np_,8,64] bf16 ping-pong; win_fn(dy,dx)->AP."""
                for t, (dy, dx) in enumerate(TAPS):
                    win = win_fn(dy, dx)
                    if t == 0:
                        eng.tensor_scalar(
                            out=out_t[:np_, :, :], in0=win,
                            scalar1=w_col(t), scalar2=None, op0=ALU.mult)
                    else:
                        dst, src = (out_t, scr_t) if t % 2 == 0 else \
                                   (scr_t, out_t)
                        eng.scalar_tensor_tensor(
                            out=dst[:np_, :, :], in0=win, scalar=w_col(t),
                            in1=src[:np_, :, :], op0=ALU.mult, op1=ALU.add)

            def pe_dw_chunk(wslot, img, out_tag, pc=128, wt=None):
                wt = wt if wt is not None else dwx_t
                ps = psxw.tile([128, 8, 64], F32, tag="psxw", name="psxw")
                for t, (dy, dx) in enumerate(TAPS):
                    nc.tensor.matmul(
                        ps[:pc, :, :],
                        wt[:pc, wslot, 128 * t:128 * t + pc],
                        img[:pc, 1 + dy:9 + dy, 1 + dx:65 + dx],
                        start=(t == 0), stop=(t == 8))
                out = sA.tile([128, 8, 64], BF16, tag=out_tag, name=out_tag)
                nc.scalar.copy(out[:pc, :, :], ps[:pc, :, :])
                return out

            def emit_A(n, ht, xps):
                r0 = 8 * n
                xw01 = [pe_dw_chunk(ci, xps[ci], f"xw{ci}") for ci in range(2)]
                xw2 = pe_dw_chunk(2, xps[2], "xw2", pc=64)

                for m in range(4):
                    ps = psi.tile([128, 512], F32, tag="psi", name="psi")
                    for k in range(4):  # h chunks first (ready earlier)
                        nc.tensor.matmul(
                            ps[:, :], wy_t[:, k, 128 * m:128 * (m + 1)],
                            ht[:, k, :], start=(k == 0), stop=False)
                    nc.tensor.matmul(
                        ps[:, :], wy_t[:, 4, 128 * m:128 * (m + 1)],
                        xw01[0][:, :, :], start=False, stop=False)
                    nc.tensor.matmul(
                        ps[:, :], wy_t[:, 5, 128 * m:128 * (m + 1)],
                        xw01[1][:, :, :], start=False, stop=False)
                    nc.tensor.matmul(
                        ps[:, :], wy_t[:64, 6, 128 * m:128 * (m + 1)],
                        xw2[0:64, :, :], start=False, stop=True)
                    nc.scalar.activation(
                        i_pad[m][:, 1 + r0:9 + r0, 1:65], ps[:, :],
                        AF.Identity, bias=wyb_t[:, m:m + 1], scale=1.0)

            def emit_dwi(n):
                r0 = 8 * n
                b_sb = []
                for ci in range(3):
                    if n == NCHUNK - 1:  # tail slab: PE is idle, DVE is not
                        b_sb.append(pe_dw_chunk(
                            ci, i_pad[ci][:, r0:r0 + 10, :], f"b{ci}",
                            wt=dwi_t))
                        continue
                    bt = sB.tile([128, 8, 64], BF16, tag=f"b{ci}",
                                 name=f"b{ci}")
                    scr = sB.tile([128, 8, 64], BF16, tag=f"bs{ci}",
                                  name=f"bs{ci}")
                    dw_chain(
                        nc.vector, bt, scr,
                        lambda dy, dx, _c=ci: i_pad[_c][
                            :, 1 + r0 + dy:9 + r0 + dy, 1 + dx:65 + dx],
                        lambda t, _c=ci: dwv_t[:, _c, t:t + 1])
                    b_sb.append(bt)
                # chunk 3 on PE (diag matmul, weight slot 3)
                b_sb.append(pe_dw_chunk(3, i_pad[3][:, r0:r0 + 10, :], "b3"))
                return b_sb

            def emit_gates(n, b_sb):
                c_t = sB.tile([128, 4, 512], BF16, tag="c", name="c")
                nc.sync.dma_start(
                    out=c_t[:],
                    in_=c_ap[:, 512 * n:512 * (n + 1)].rearrange(
                        "(k p) x -> p k x", p=128))
                occ_t = sB.tile([128, 4, 512], BF16, tag="occ", name="occ")
                och_t = sB.tile([128, 4, 512], BF16, tag="och", name="och")
                for m in range(4):
                    sig = []
                    for g in range(4):  # 0=i 1=f 2=c 3=o
                        ps = psg.tile([128, 512], F32, tag="psg", name="psg")
                        for k in range(4):
                            nc.tensor.matmul(
                                ps[:, :],
                                wg_t[:, 4 * g + k, 128 * m:128 * (m + 1)],
                                b_sb[k][:, :, :],
                                start=(k == 0), stop=(k == 3))
                        st = sB.tile([128, 512], BF16, tag=f"sg{g}",
                                     name=f"sg{g}")
                        nc.scalar.activation(
                            st[:, :], ps[:, :],
                            AF.Relu if g == 2 else AF.Sigmoid)
                        sig.append(st)

                    u1 = sB.tile([128, 512], BF16, tag="u1", name="u1")
                    nc.vector.tensor_mul(u1[:, :], sig[1][:, :], c_t[:, m, :])
                    u2 = sB.tile([128, 512], BF16, tag="u2", name="u2")
                    nc.vector.scalar_tensor_tensor(
                        out=u2[:, :], in0=sig[2][:, :], scalar=6.0,
                        in1=sig[0][:, :], op0=ALU.min, op1=ALU.mult)
                    nc.vector.tensor_add(occ_t[:, m, :], u1[:, :], u2[:, :])
                    rcc = sB.tile([128, 512], BF16, tag="rcc", name="rcc")
                    nc.vector.tensor_scalar(
                        out=rcc[:, :], in0=occ_t[:, m, :],
                        scalar1=0.0, scalar2=6.0, op0=ALU.max, op1=ALU.min)
                    nc.vector.tensor_mul(och_t[:, m, :], rcc[:, :],
                                         sig[3][:, :])
                    if n == NCHUNK - 1:
                        # tail slab: drain each m as soon as it is ready
                        nc.sync.dma_start(
                            out=cc_ap[128 * m:128 * (m + 1),
                                      512 * n:512 * (n + 1)],
                            in_=occ_t[:, m, :])
                        nc.sync.dma_start(
                            out=ch_ap[128 * m:128 * (m + 1),
                                      512 * n:512 * (n + 1)],
                            in_=och_t[:, m, :])
                if n < NCHUNK - 1:
                    nc.sync.dma_start(
                        out=cc_ap[:, 512 * n:512 * (n + 1)].rearrange(
                            "(k p) x -> p k x", p=128),
                        in_=occ_t[:])
                    nc.sync.dma_start(
                        out=ch_ap[:, 512 * n:512 * (n + 1)].rearrange(
                            "(k p) x -> p k x", p=128),
                        in_=och_t[:])

            # ---- software pipeline ----
            bq = {}
            for n in range(NCHUNK + 2):
                if n < NCHUNK:
                    if n not in ins:
                        ins[n] = emit_slab_inputs(n)
                    emit_A(n, *ins.pop(n))
                    if n + 1 < NCHUNK and n + 1 not in ins:
                        ins[n + 1] = emit_slab_inputs(n + 1)
                if 1 <= n <= NCHUNK:
                    bq[n - 1] = emit_dwi(n - 1)
                if n >= 2:
                    emit_gates(n - 2, bq.pop(n - 2))

    nc.compile()
    return nc


def pack_weights(W_dw, W_dwb, Wy, Wy_b, Wi, Wbi, Wbf, Wbc, Wbo):
    WyT = Wy[:, :, 0, 0].T.astype(np.float32)  # (832, 512) lhsT
    wy = np.zeros((128, 7, 512), np.float32)
    for k in range(4):  # h chunks first
        wy[:, k, :] = WyT[320 + 128 * k:320 + 128 * (k + 1), :]
    for k in range(2):
        wy[:, 4 + k, :] = WyT[128 * k:128 * (k + 1), :]
    wy[:64, 6, :] = WyT[256:320, :]

    wyb = (Wy_b + Wy[:, :320, 0, 0] @ W_dwb).astype(np.float32)
    wyb = wyb.reshape(4, 128).T.copy()

    wdx = W_dw[:, 0].reshape(CIN, 9)
    wdi = Wi[:, 0].reshape(CH, 9)
    dwx = np.zeros((128, 4, 1152), np.float32)
    for slot, w in enumerate([wdx[0:128], wdx[128:256], wdx[256:320],
                              wdi[384:512]]):
        pc = w.shape[0]
        for t in range(9):
            idx = np.arange(pc)
            dwx[idx, slot, 128 * t + idx] = w[idx, t]

    dwv = np.zeros((128, 5, 9), np.float32)
    for ci in range(4):
        dwv[:, ci, :] = wdi[128 * ci:128 * (ci + 1), :]
    dwi = np.zeros((128, 3, 1152), np.float32)
    for ci in range(3):
        for t in range(9):
            idx = np.arange(128)
            dwi[idx, ci, 128 * t + idx] = wdi[128 * ci + idx, t]

    wg = np.zeros((128, 16, 512), np.float32)
    for g, W in enumerate([Wbi, Wbf, Wbc, Wbo]):
        lhsT = W[:, :, 0, 0].T.astype(np.float32)  # (512 in, 512 out)
        for k in range(4):
            wg[:, 4 * g + k, :] = lhsT[128 * k:128 * (k + 1), :]

    bf = lambda a: np.ascontiguousarray(a).astype(NPBF16)
    return {
        "wy": bf(wy), "wyb": np.ascontiguousarray(wyb), "wg": bf(wg),
        "dwx": bf(dwx), "dwi": bf(dwi),
        "dwv": np.ascontiguousarray(dwv, dtype=np.float32),
    }


_CACHE = {}


def _get_nc():
    if "nc" not in _CACHE:
        _CACHE["nc"] = build_nc()
    return _CACHE["nc"]


def run(inputs, trace=False, tmpdir=None):
    """inputs: dict as from setup_inputs(). Returns ((ch, cc), results_obj)."""
    inp = {k: np.asarray(v, np.float32) for k, v in inputs.items()}
    packed = pack_weights(
        inp["W_dw"], inp["W_dwb"], inp["Wy"], inp["Wy_b"], inp["Wi"],
        inp["Wbi"], inp["Wbf"], inp["Wbc"], inp["Wbo"],
    )
    xpad_host = np.zeros((B, CIN, 66, 66), NPBF16)
    xpad_host[:, :, 1:65, 1:65] = inp["x"].astype(NPBF16)
    h_host = inp["h"].reshape(B, CH, PIX).astype(NPBF16)
    c_host = inp["c"].reshape(B, CH, PIX).astype(NPBF16)
    in_maps = []
    for b in range(B):
        in_maps.append({
            "x": xpad_host[b],
            "h": np.ascontiguousarray(h_host[b]),
            "c": np.ascontiguousarray(c_host[b]),
            **packed,
        })
    nc = _get_nc()
    kwargs = {}
    if trace:
        _enable_trace_hooks()
        kwargs = dict(trace=True, trace_cores=[0])
        if tmpdir:
            kwargs["tmpdir"] = tmpdir
    res = run_bass_kernel_spmd(nc, in_maps, core_ids=list(range(NCORES)), **kwargs)
    ch = np.stack([res.results[b]["och"].astype(np.float32).reshape(CH, HW, HW)
                   for b in range(B)])
    cc = np.stack([res.results[b]["occ"].astype(np.float32).reshape(CH, HW, HW)
                   for b in range(B)])
    return (ch, cc), res


def kernel(**inputs):
    (ch, cc), _ = run(inputs, trace=False)
    return ch, cc


# ---------- optional NTFF tracing support (test harness only) ----------

def _enable_trace_hooks():
    import types, ctypes, contextlib
    if "antenv.axon_hooks" in sys.modules:
        return
    import concourse.bass_utils as bass_utils

    def _ntff_profile_via_ctypes(so_path):
        lib = ctypes.CDLL(so_path)
        if not hasattr(lib, "axon_start_nrt_profile"):
            return None
        lib.axon_start_nrt_profile.argtypes = [
            ctypes.POINTER(ctypes.c_int64), ctypes.c_size_t]
        lib.axon_start_nrt_profile.restype = ctypes.c_int64
        lib.axon_stop_nrt_profile.argtypes = [ctypes.c_char_p]
        lib.axon_stop_nrt_profile.restype = ctypes.c_int64

        @contextlib.contextmanager
        def _hook(output_dir, device_ids):
            import jax
            jax.devices()
            if device_ids:
                ids = (ctypes.c_int64 * len(device_ids))(*device_ids)
                rc = lib.axon_start_nrt_profile(ids, len(device_ids))
            else:
                rc = lib.axon_start_nrt_profile(None, 0)
            if rc != 0:
                raise RuntimeError(f"axon_start_nrt_profile rc={rc}")
            try:
                yield
            finally:
                lib.axon_stop_nrt_profile(str(output_dir).encode())
        return _hook

    hook = _ntff_profile_via_ctypes("/opt/axon/libaxon_pjrt.so")
    mod = types.ModuleType("antenv.axon_hooks")
    mod.get_axon_ntff_profile_hook = lambda: hook
    mod.set_axon_ntff_profile_hook = lambda h: None
    sys.modules["antenv.axon_hooks"] = mod
    bass_utils.upload_artifacts = lambda tmpdir: "local://" + str(tmpdir)


# revision 30
# speedup vs baseline: 1.2117x; 1.0004x over previous
"""BottleneckLSTMCell fused kernel for 8 Trainium2 NeuronCores.

Sharding: data-parallel over batch (B=8 -> 1 image per core). Each core runs
the full cell for its image:

  phase A: xw = dw3x3(x) (+bias folded into the Wy bias); i = Wy @ [h; xw] + b
  phase B: b = dw3x3(i); four 1x1 gate matmuls; LSTM pointwise -> (ch, cc)

All matmul operands are bf16 (PSUM accumulates fp32). The tensor engine keeps
the dense 1x1 matmuls plus two depthwise chunks; the remaining five depthwise
chunks run as per-partition-scalar multiply-add chains on the vector engine
(bf16 2x mode) and gpsimd, which are otherwise idle. Phases are
software-pipelined: gates of slab n overlap dw-i of slab n+1 and phase A of
slab n+2.
"""

import sys

if '/opt/trn_rl_repo' not in sys.path:
    sys.path.insert(0, '/opt/trn_rl_repo')

import numpy as np
import ml_dtypes

import concourse.bass as bass  # noqa: F401
from concourse import bacc
import concourse.mybir as mybir
from concourse.tile import TileContext
from concourse.bass_utils import run_bass_kernel_spmd

F32 = mybir.dt.float32
BF16 = mybir.dt.bfloat16
NPBF16 = ml_dtypes.bfloat16
AF = mybir.ActivationFunctionType
ALU = mybir.AluOpType

B, CIN, CH, HW = 8, 320, 512, 64
PIX = HW * HW          # 4096
NCORES = 8
NCHUNK = 8             # spatial slabs of 8 rows (512 px)

TAPS = [(t // 3 - 1, t % 3 - 1) for t in range(9)]


def build_nc():
    nc = bacc.Bacc(None, target_bir_lowering=False, num_devices=NCORES)

    xd = nc.dram_tensor("x", (CIN, 66, 66), BF16, kind="ExternalInput")
    hd = nc.dram_tensor("h", (CH, PIX), BF16, kind="ExternalInput")
    cd = nc.dram_tensor("c", (CH, PIX), BF16, kind="ExternalInput")
    wyd = nc.dram_tensor("wy", (128, 7, 512), BF16, kind="ExternalInput")
    wybd = nc.dram_tensor("wyb", (128, 4), F32, kind="ExternalInput")
    wgd = nc.dram_tensor("wg", (128, 16, 512), BF16, kind="ExternalInput")
    # diag-packed PE dw weights: slots 0,1 = dwx chunks 0,1; slot 2 = dwx
    # chunk2 (64ch); slot 3 = dwi chunk 3
    dwxd = nc.dram_tensor("dwx", (128, 4, 1152), BF16, kind="ExternalInput")
    # per-partition tap weights: [:, 0:4] = dwi chunks, [:, 4] = dwx chunk2
    dwvd = nc.dram_tensor("dwv", (128, 5, 9), F32, kind="ExternalInput")
    # diag-packed dwi chunks 0..2 for the PE tail slab
    dwid = nc.dram_tensor("dwi", (128, 3, 1152), BF16, kind="ExternalInput")
    ccd = nc.dram_tensor("occ", (CH, PIX), BF16, kind="ExternalOutput")
    chd = nc.dram_tensor("och", (CH, PIX), BF16, kind="ExternalOutput")

    x_ap, h_ap, c_ap = xd.ap(), hd.ap(), cd.ap()
    cc_ap, ch_ap = ccd.ap(), chd.ap()

    with TileContext(nc) as tc:
        with (
            tc.tile_pool(name="persist", bufs=1) as pp,
            tc.tile_pool(name="sA", bufs=2) as sA,
            tc.tile_pool(name="sB", bufs=2) as sB,
            tc.tile_pool(name="psxw", bufs=2, space="PSUM") as psxw,
            tc.tile_pool(name="psi", bufs=2, space="PSUM") as psi,
            tc.tile_pool(name="psg", bufs=4, space="PSUM") as psg,
        ):
            # per-partition dw tap weights -- first on the vector queue so
            # the slab-0 DVE chains are unblocked immediately
            dwv_t = pp.tile([128, 5, 9], F32, tag="dwv", name="dwv")
            nc.gpsimd.dma_start(out=dwv_t[:], in_=dwvd.ap())

            i_pad = [pp.tile([128, 66, 66], BF16, tag=f"ipad{m}",
                             name=f"ipad{m}") for m in range(4)]
            for m in range(4):
                nc.vector.memset(i_pad[m][:, 0, :], 0.0)
                nc.vector.memset(i_pad[m][:, 65, :], 0.0)
                nc.vector.memset(i_pad[m][:, :, 0], 0.0)
                nc.vector.memset(i_pad[m][:, :, 65], 0.0)

            # PE diag weights -- dwx slots first (startup critical); the
            # dwi3 slot is not needed until the first emit_dwi
            dwx_t = pp.tile([128, 4, 1152], BF16, tag="dwx", name="dwx")
            with tc.high_priority():
                for _s in range(3):
                    nc.sync.dma_start(out=dwx_t[:, _s, :],
                                      in_=dwxd.ap()[:, _s, :])
            nc.scalar.dma_start(out=dwx_t[:, 3, :], in_=dwxd.ap()[:, 3, :])

            def emit_slab_inputs(n, q=None):
                q = q or nc.sync
                r0 = 8 * n
                xps = []
                for ci in range(3):
                    pc = 128 if ci < 2 else 64
                    xp = sA.tile([128, 10, 66], BF16, tag=f"xpad{ci}",
                                 name=f"xpad{ci}")
                    q.dma_start(
                        out=xp[:pc, :, :],
                        in_=x_ap[128 * ci:128 * ci + pc, r0:r0 + 10, :])
                    xps.append(xp)
                ht = sA.tile([128, 4, 512], BF16, tag="h", name="h")
                q.dma_start(
                    out=ht[:],
                    in_=h_ap[:, 512 * n:512 * (n + 1)].rearrange(
                        "(k p) x -> p k x", p=128))
                return ht, xps

            with tc.high_priority():
                ins = {0: emit_slab_inputs(0, q=nc.gpsimd)}

            # bulk weights off the critical queue (scalar engine is idle now)
            wy_t = pp.tile([128, 7, 512], BF16, tag="wy", name="wy")
            nc.scalar.dma_start(out=wy_t[:], in_=wyd.ap())
            wyb_t = pp.tile([128, 4], F32, tag="wyb", name="wyb")
            nc.scalar.dma_start(out=wyb_t[:], in_=wybd.ap())
            wg_t = pp.tile([128, 16, 512], BF16, tag="wg", name="wg")
            nc.scalar.dma_start(out=wg_t[:], in_=wgd.ap())
            dwi_t = pp.tile([128, 3, 1152], BF16, tag="dwi", name="dwi")
            nc.scalar.dma_start(out=dwi_t[:], in_=dwid.ap())

            ins[1] = emit_slab_inputs(1)

            def dw_chain(eng, out_t, scr_t, win_fn, w_col, np_=128):
                """9-tap depthwise on an elementwise engine.
                out_t/scr_t: [np_,8,64] bf16 ping-pong; win_fn(dy,dx)->AP."""
                for t, (dy, dx) in enumerate(TAPS):
                    win = win_fn(dy, dx)
                    if t == 0:
                        eng.tensor_scalar(
                            out=out_t[:np_, :, :], in0=win,
                            scalar1=w_col(t), scalar2=None, op0=ALU.mult)
                    else:
                        dst, src = (out_t, scr_t) if t % 2 == 0 else \
                                   (scr_t, out_t)
                        eng.scalar_tensor_tensor(
                            out=dst[:np_, :, :], in0=win, scalar=w_col(t),
                            in1=src[:np_, :, :], op0=ALU.mult, op1=ALU.add)

            def pe_dw_chunk(wslot, img, out_tag, pc=128, wt=None):
                wt = wt if wt is not None else dwx_t
                ps = psxw.tile([128, 8, 64], F32, tag="psxw", name="psxw")
                for t, (dy, dx) in enumerate(TAPS):
                    nc.tensor.matmul(
                        ps[:pc, :, :],
                        wt[:pc, wslot, 128 * t:128 * t + pc],
                        img[:pc, 1 + dy:9 + dy, 1 + dx:65 + dx],
                        start=(t == 0), stop=(t == 8))
                out = sA.tile([128, 8, 64], BF16, tag=out_tag, name=out_tag)
                nc.scalar.copy(out[:pc, :, :], ps[:pc, :, :])
                return out

            def emit_A(n, ht, xps):
                r0 = 8 * n
                xw01 = [pe_dw_chunk(ci, xps[ci], f"xw{ci}") for ci in range(2)]
                xw2 = pe_dw_chunk(2, xps[2], "xw2", pc=64)

                for m in range(4):
                    ps = psi.tile([128, 512], F32, tag="psi", name="psi")
                    for k in range(4):  # h chunks first (ready earlier)
                        nc.tensor.matmul(
                            ps[:, :], wy_t[:, k, 128 * m:128 * (m + 1)],
                            ht[:, k, :], start=(k == 0), stop=False)
                    nc.tensor.matmul(
                        ps[:, :], wy_t[:, 4, 128 * m:128 * (m + 1)],
                        xw01[0][:, :, :], start=False, stop=False)
                    nc.tensor.matmul(
                        ps[:, :], wy_t[:, 5, 128 * m:128 * (m + 1)],
                        xw01[1][:, :, :], start=False, stop=False)
                    nc.tensor.matmul(
                        ps[:, :], wy_t[:64, 6, 128 * m:128 * (m + 1)],
                        xw2[0:64, :, :], start=False, stop=True)
                    nc.scalar.activation(
                        i_pad[m][:, 1 + r0:9 + r0, 1:65], ps[:, :],
                        AF.Identity, bias=wyb_t[:, m:m + 1], scale=1.0)

            def emit_dwi(n):
                r0 = 8 * n
                b_sb = []
                for ci in range(3):
                    if n == NCHUNK - 1:  # tail slab: PE is idle, DVE is not
                        b_sb.append(pe_dw_chunk(
                            ci, i_pad[ci][:, r0:r0 + 10, :], f"b{ci}",
                            wt=dwi_t))
                        continue
                    bt = sB.tile([128, 8, 64], BF16, tag=f"b{ci}",
                                 name=f"b{ci}")
                    scr = sB.tile([128, 8, 64], BF16, tag=f"bs{ci}",
                                  name=f"bs{ci}")
                    dw_chain(
                        nc.vector, bt, scr,
                        lambda dy, dx, _c=ci: i_pad[_c][
                            :, 1 + r0 + dy:9 + r0 + dy, 1 + dx:65 + dx],
                        lambda t, _c=ci: dwv_t[:, _c, t:t + 1])
                    b_sb.append(bt)
                # chunk 3 on PE (diag matmul, weight slot 3)
                b_sb.append(pe_dw_chunk(3, i_pad[3][:, r0:r0 + 10, :], "b3"))
                return b_sb

            def emit_gates(n, b_sb):
                c_t = sB.tile([128, 4, 512], BF16, tag="c", name="c")
                nc.sync.dma_start(
                    out=c_t[:],
                    in_=c_ap[:, 512 * n:512 * (n + 1)].rearrange(
                        "(k p) x -> p k x", p=128))
                occ_t = sB.tile([128, 4, 512], BF16, tag="occ", name="occ")
                och_t = sB.tile([128, 4, 512], BF16, tag="och", name="och")
                for m in range(4):
                    sig = []
                    for g in range(4):  # 0=i 1=f 2=c 3=o
                        ps = psg.tile([128, 512], F32, tag="psg", name="psg")
                        for k in range(4):
                            nc.tensor.matmul(
                                ps[:, :],
                                wg_t[:, 4 * g + k, 128 * m:128 * (m + 1)],
                                b_sb[k][:, :, :],
                                start=(k == 0), stop=(k == 3))
                        st = sB.tile([128, 512], BF16, tag=f"sg{g}",
                                     name=f"sg{g}")
                        nc.scalar.activation(
                            st[:, :], ps[:, :],
                            AF.Relu if g == 2 else AF.Sigmoid)
                        sig.append(st)

                    u1 = sB.tile([128, 512], BF16, tag="u1", name="u1")
                    nc.vector.tensor_mul(u1[:, :], sig[1][:, :], c_t[:, m, :])
                    u2 = sB.tile([128, 512], BF16, tag="u2", name="u2")
                    nc.vector.scalar_tensor_tensor(
                        out=u2[:, :], in0=sig[2][:, :], scalar=6.0,
                        in1=sig[0][:, :], op0=ALU.min, op1=ALU.mult)
                    nc.vector.tensor_add(occ_t[:, m, :], u1[:, :], u2[:, :])
                    rcc = sB.tile([128, 512], BF16, tag="rcc", name="rcc")
                    nc.vector.tensor_scalar(
                        out=rcc[:, :], in0=occ_t[:, m, :],
                        scalar1=0.0, scalar2=6.0, op0=ALU.max, op1=ALU.min)
                    nc.vector.tensor_mul(och_t[:, m, :], rcc[:, :],
                                         sig[3][:, :])
                    if n == NCHUNK - 1:
                        # tail slab: drain each m as soon as it is ready
                        nc.sync.dma_start(
                            out=cc_ap[128 * m:128 * (m + 1),
                                      512 * n:512 * (n + 1)],
                            in_=occ_t[:, m, :])
                        nc.sync.dma_start(
                            out=ch_ap[128 * m:128 * (m + 1),
                                      512 * n:512 * (n + 1)],
                            in_=och_t[:, m, :])
                if n < NCHUNK - 1:
                    nc.sync.dma_start(
                        out=cc_ap[:, 512 * n:512 * (n + 1)].rearrange(
                            "(k p) x -> p k x", p=128),
                        in_=occ_t[:])
                    nc.sync.dma_start(
                        out=ch_ap[:, 512 * n:512 * (n + 1)].rearrange(
                            "(k p) x -> p k x", p=128),
                        in_=och_t[:])

            # ---- software pipeline ----
            bq = {}
            for n in range(NCHUNK + 2):
                if n < NCHUNK:
                    if n not in ins:
                        ins[n] = emit_slab_inputs(n)
                    emit_A(n, *ins.pop(n))
                    if n + 1 < NCHUNK and n + 1 not in ins:
                        ins[n + 1] = emit_slab_inputs(n + 1)
                if 1 <= n <= NCHUNK:
                    bq[n - 1] = emit_dwi(n - 1)
                if n >= 2:
                    emit_gates(n - 2, bq.pop(n - 2))

    nc.compile()
    return nc


def pack_weights(W_dw, W_dwb, Wy, Wy_b, Wi, Wbi, Wbf, Wbc, Wbo):
    WyT = Wy[:, :, 0, 0].T.astype(np.float32)  # (832, 512) lhsT
    wy = np.zeros((128, 7, 512), np.float32)
    for k in range(4):  # h chunks first
        wy[:, k, :] = WyT[320 + 128 * k:320 + 128 * (k + 1), :]
    for k in range(2):
        wy[:, 4 + k, :] = WyT[128 * k:128 * (k + 1), :]
    wy[:64, 6, :] = WyT[256:320, :]

    wyb = (Wy_b + Wy[:, :320, 0, 0] @ W_dwb).astype(np.float32)
    wyb = wyb.reshape(4, 128).T.copy()

    wdx = W_dw[:, 0].reshape(CIN, 9)
    wdi = Wi[:, 0].reshape(CH, 9)
    dwx = np.zeros((128, 4, 1152), np.float32)
    for slot, w in enumerate([wdx[0:128], wdx[128:256], wdx[256:320],
                              wdi[384:512]]):
        pc = w.shape[0]
        for t in range(9):
            idx = np.arange(pc)
            dwx[idx, slot, 128 * t + idx] = w[idx, t]

    dwv = np.zeros((128, 5, 9), np.float32)
    for ci in range(4):
        dwv[:, ci, :] = wdi[128 * ci:128 * (ci + 1), :]
    dwi = np.zeros((128, 3, 1152), np.float32)
    for ci in range(3):
        for t in range(9):
            idx = np.arange(128)
            dwi[idx, ci, 128 * t + idx] = wdi[128 * ci + idx, t]

    wg = np.zeros((128, 16, 512), np.float32)
    for g, W in enumerate([Wbi, Wbf, Wbc, Wbo]):
        lhsT = W[:, :, 0, 0].T.astype(np.float32)  # (512 in, 512 out)
        for k in range(4):
            wg[:, 4 * g + k, :] = lhsT[128 * k:128 * (k + 1), :]

    bf = lambda a: np.ascontiguousarray(a).astype(NPBF16)
    return {
        "wy": bf(wy), "wyb": np.ascontiguousarray(wyb), "wg": bf(wg),
        "dwx": bf(dwx), "dwi": bf(dwi),
        "dwv": np.ascontiguousarray(dwv, dtype=np.float32),
    }


_CACHE = {}


def _get_nc():
    if "nc" not in _CACHE:
        _CACHE["nc"] = build_nc()
    return _CACHE["nc"]


def run(inputs, trace=False, tmpdir=None):
    """inputs: dict as from setup_inputs(). Returns ((ch, cc), results_obj)."""
    inp = {k: np.asarray(v, np.float32) for k, v in inputs.items()}
    packed = pack_weights(
        inp["W_dw"], inp["W_dwb"], inp["Wy"], inp["Wy_b"], inp["Wi"],
        inp["Wbi"], inp["Wbf"], inp["Wbc"], inp["Wbo"],
    )
    xpad_host = np.zeros((B, CIN, 66, 66), NPBF16)
    xpad_host[:, :, 1:65, 1:65] = inp["x"].astype(NPBF16)
    h_host = inp["h"].reshape(B, CH, PIX).astype(NPBF16)
    c_host = inp["c"].reshape(B, CH, PIX).astype(NPBF16)
    in_maps = []
    for b in range(B):
        in_maps.append({
            "x": xpad_host[b],
            "h": np.ascontiguousarray(h_host[b]),
            "c": np.ascontiguousarray(c_host[b]),
            **packed,
        })
    nc = _get_nc()
    kwargs = {}
    if trace:
        _enable_trace_hooks()
        kwargs = dict(trace=True, trace_cores=[0])
        if tmpdir:
            kwargs["tmpdir"] = tmpdir
    res = run_bass_kernel_spmd(nc, in_maps, core_ids=list(range(NCORES)), **kwargs)
    ch = np.stack([res.results[b]["och"].astype(np.float32).reshape(CH, HW, HW)
                   for b in range(B)])
    cc = np.stack([res.results[b]["occ"].astype(np.float32).reshape(CH, HW, HW)
                   for b in range(B)])
    return (ch, cc), res


def kernel(**inputs):
    (ch, cc), _ = run(inputs, trace=False)
    return ch, cc


# ---------- optional NTFF tracing support (test harness only) ----------

def _enable_trace_hooks():
    import types, ctypes, contextlib
    if "antenv.axon_hooks" in sys.modules:
        return
    import concourse.bass_utils as bass_utils

    def _ntff_profile_via_ctypes(so_path):
        lib = ctypes.CDLL(so_path)
        if not hasattr(lib, "axon_start_nrt_profile"):
            return None
        lib.axon_start_nrt_profile.argtypes = [
            ctypes.POINTER(ctypes.c_int64), ctypes.c_size_t]
        lib.axon_start_nrt_profile.restype = ctypes.c_int64
        lib.axon_stop_nrt_profile.argtypes = [ctypes.c_char_p]
        lib.axon_stop_nrt_profile.restype = ctypes.c_int64

        @contextlib.contextmanager
        def _hook(output_dir, device_ids):
            import jax
            jax.devices()
            if device_ids:
                ids = (ctypes.c_int64 * len(device_ids))(*device_ids)
                rc = lib.axon_start_nrt_profile(ids, len(device_ids))
            else:
                rc = lib.axon_start_nrt_profile(None, 0)
            if rc != 0:
                raise RuntimeError(f"axon_start_nrt_profile rc={rc}")
            try:
                yield
            finally:
                lib.axon_stop_nrt_profile(str(output_dir).encode())
        return _hook

    hook = _ntff_profile_via_ctypes("/opt/axon/libaxon_pjrt.so")
    mod = types.ModuleType("antenv.axon_hooks")
    mod.get_axon_ntff_profile_hook = lambda: hook
    mod.set_axon_ntff_profile_hook = lambda h: None
    sys.modules["antenv.axon_hooks"] = mod
    bass_utils.upload_artifacts = lambda tmpdir: "local://" + str(tmpdir)
